# revision 1
# baseline (speedup 1.0000x reference)
"""AdaptiveGraphPooling Trainium2 kernel v2 (8 NeuronCores, SPMD).

Layout: graphs paired (2p, 2p+1); pairs sorted by padded length, grouped in
blocks of NB=16 with block-uniform slot length LB (mult of 32). Within a
block, columns are position-major: col = off_b + pos*NB + j. Two DRAM
tensors: xlo[128, W] (rows 0:64 = graph-A feats 0:64, rows 64:128 = graph-B
feats 0:64), xhi likewise for feats 64:128.

Per chunk of 1024 cols:
  PE: h-pair psum via 2 block-diag matmuls/512; s-bcast via 1 rank-1
      block-diag matmul/512; z-extract rows {0,64} of eps accumulated in a
      [2,512] psum; segment sums via identity-matmul accumulation into
      [128,512] psums (exact fp32).
  ACT: tanh (1024-wide, bias), exp (1024-wide) -> eps bf16 SBUF.
  DVE: y = x*eps (TT mult 2x), streaming TT-max / TT-add merges into
      [128,1024] position-folded accumulators.
  Pool: tail folds 1024->16 per block; final small copies.
Final: assemble [128 feat, 128 graph] pool tiles (A-graphs | B-graphs in
sorted order), normalize, out-proj, selector MLP, softmax-3 mix, DMA out.
Host un-permutes rows. No collectives.
"""

import numpy as np
import ml_dtypes

from concourse import bacc, mybir
from concourse import tile as tile_mod
from concourse.bass_utils import run_bass_kernel_spmd

BF16 = mybir.dt.bfloat16
F32 = mybir.dt.float32
ALU = mybir.AluOpType
ACTF = mybir.ActivationFunctionType

D = 128
A = 64
NCORES = 8
NB = 16            # pairs per block
NPAIR = 128        # pairs per core
NBLK = NPAIR // NB  # 8 blocks


def build_nc(LBs, W, variant="full"):
    """SPMD program. LBs: tuple of 8 block slot lengths (mult of 32)."""
    LBs = tuple(int(v) for v in LBs)
    assert len(LBs) == NBLK and all(v % 32 == 0 for v in LBs)
    offs = np.concatenate([[0], np.cumsum([NB * lb for lb in LBs])])
    assert offs[-1] == W

    nc = bacc.Bacc(None, target_bir_lowering=False, debug=False)

    xloP = nc.declare_dram_parameter("xlo", [D, W], BF16, isOutput=False)
    xhiP = nc.declare_dram_parameter("xhi", [D, W], BF16, isOutput=False)
    BDWloP = nc.declare_dram_parameter("BDWlo", [D, D], BF16, isOutput=False)
    BDWhiP = nc.declare_dram_parameter("BDWhi", [D, D], BF16, isOutput=False)
    BDctxP = nc.declare_dram_parameter("BDctx", [D, D], BF16, isOutput=False)
    biasP = nc.declare_dram_parameter("biasp", [D, 1], F32, isOutput=False)
    selEP = nc.declare_dram_parameter("selE", [D, 2], BF16, isOutput=False)
    identP = nc.declare_dram_parameter("ident", [D, D], BF16, isOutput=False)
    identfP = nc.declare_dram_parameter("identf", [D, D], F32, isOutput=False)
    outWP = nc.declare_dram_parameter("outW", [D, D], BF16, isOutput=False)
    outbP = nc.declare_dram_parameter("outb", [D, 1], F32, isOutput=False)
    selW1P = nc.declare_dram_parameter("selW1", [3 * D, D], BF16,
                                       isOutput=False)
    selb1P = nc.declare_dram_parameter("selb1", [D, 1], F32, isOutput=False)
    selW2P = nc.declare_dram_parameter("selW2", [D, 3], BF16, isOutput=False)
    selb2P = nc.declare_dram_parameter("selb2", [3, 1], F32, isOutput=False)
    countsP = nc.declare_dram_parameter("counts", [2 * NPAIR, 1], F32,
                                        isOutput=False)
    zcorrP = nc.declare_dram_parameter("zcorr", [2 * NPAIR, 1], F32,
                                       isOutput=False)
    outP = nc.declare_dram_parameter("out", [2 * NPAIR, D], F32,
                                     isOutput=True)

    with tile_mod.TileContext(nc) as tc:
        with tc.tile_pool(name="const", bufs=1) as cp:
            BDWlo = cp.tile([D, D], BF16, name="BDWlo")
            nc.sync.dma_start(BDWlo[:], BDWloP[:])
            BDWhi = cp.tile([D, D], BF16, name="BDWhi")
            nc.sync.dma_start(BDWhi[:], BDWhiP[:])
            BDctx = cp.tile([D, D], BF16, name="BDctx")
            nc.sync.dma_start(BDctx[:], BDctxP[:])
            biasp = cp.tile([D, 1], F32, name="biasp")
            nc.sync.dma_start(biasp[:], biasP[:])
            selE = cp.tile([D, 2], BF16, name="selE")
            nc.sync.dma_start(selE[:], selEP[:])
            ident = cp.tile([D, D], BF16, name="ident")
            nc.sync.dma_start(ident[:], identP[:])
            identf = cp.tile([D, D], F32, name="identf")
            nc.sync.dma_start(identf[:], identfP[:])
            outW_sb = cp.tile([D, D], BF16, name="outWsb")
            nc.sync.dma_start(outW_sb[:], outWP[:])
            outb_sb = cp.tile([D, 1], F32, name="outbsb")
            nc.sync.dma_start(outb_sb[:], outbP[:])
            selW1_sb = cp.tile([D, 3 * D], BF16, name="selW1sb")
            for k in range(3):
                nc.sync.dma_start(selW1_sb[:, k * D:(k + 1) * D],
                                  selW1P[k * D:(k + 1) * D, :])
            selb1_sb = cp.tile([D, 1], F32, name="selb1sb")
            nc.sync.dma_start(selb1_sb[:], selb1P[:])
            selW2_sb = cp.tile([D, 3], BF16, name="selW2sb")
            nc.sync.dma_start(selW2_sb[:], selW2P[:])
            selb2_sb = cp.tile([3, 1], F32, name="selb2sb")
            nc.sync.dma_start(selb2_sb[:], selb2P[:])
            counts_bl = []
            zcorr_bl = []
            for g in range(2):
                cb = cp.tile([128, 1], F32, name=f"counts{g}")
                nc.sync.dma_start(cb[:], countsP[g * 128:(g + 1) * 128, :])
                counts_bl.append(cb)
                zb = cp.tile([128, 1], F32, name=f"zcorr{g}")
                nc.sync.dma_start(zb[:], zcorrP[g * 128:(g + 1) * 128, :])
                zcorr_bl.append(zb)

            # per-block [D,256] stashes (cols = 16 pos x 16 pairs), folded
            # globally after the block loop; final tiles cols = pair index
            sumloS = cp.tile([D, 2048], BF16, name="sumloS")
            sumhiS = cp.tile([D, 2048], BF16, name="sumhiS")
            maxloS = cp.tile([D, 2048], BF16, name="maxloS")
            maxhiS = cp.tile([D, 2048], BF16, name="maxhiS")
            exloS = cp.tile([D, 2048], BF16, name="exloS")
            exhiS = cp.tile([D, 2048], BF16, name="exhiS")
            sumloC = cp.tile([D, NPAIR], F32, name="sumloC")
            sumhiC = cp.tile([D, NPAIR], F32, name="sumhiC")
            maxloC = cp.tile([D, NPAIR], BF16, name="maxloC")
            maxhiC = cp.tile([D, NPAIR], BF16, name="maxhiC")
            exloC = cp.tile([D, NPAIR], F32, name="exloC")
            exhiC = cp.tile([D, NPAIR], F32, name="exhiC")
            zC = cp.tile([2, NPAIR], F32, name="zC")

            with (
                tc.tile_pool(name="xs", bufs=2) as xpool,
                tc.tile_pool(name="th", bufs=2) as thpool,
                tc.tile_pool(name="eps", bufs=2) as epool,
                tc.tile_pool(name="ys", bufs=3) as ypool,
                tc.tile_pool(name="accs", bufs=1) as apool,
                tc.tile_pool(name="fold", bufs=2) as fpool,
                tc.tile_pool(name="hp", bufs=1, space="PSUM") as hpp,
                tc.tile_pool(name="sp", bufs=1, space="PSUM") as spp,
                tc.tile_pool(name="sums", bufs=1, space="PSUM") as smp,
                tc.tile_pool(name="zp", bufs=1, space="PSUM") as zpp,
            ):
                # block accumulators (ping-pong handled per chunk index)
                accw = 2048
                maxlo_a = [apool.tile([D, accw], BF16, name=f"maxlo{i}")
                           for i in range(2)]
                maxhi_a = [apool.tile([D, accw], BF16, name=f"maxhi{i}")
                           for i in range(2)]
                exhi_a = [apool.tile([D, accw], BF16, name=f"exhi{i}")
                         for i in range(2)]

                for b in range(NBLK):
                    LB = LBs[b]
                    CB = NB * LB
                    o0 = int(offs[b])
                    xlo_t = xpool.tile([D, CB], BF16, tag="xlo",
                                       name=f"xlo{b}")
                    nc.sync.dma_start(xlo_t[:], xloP[:, o0:o0 + CB])
                    xhi_t = xpool.tile([D, CB], BF16, tag="xhi",
                                       name=f"xhi{b}")
                    nc.scalar.dma_start(xhi_t[:], xhiP[:, o0:o0 + CB])

                    sumlo_ps = smp.tile([D, 512], F32, tag="sumlo")
                    sumhi_ps = smp.tile([D, 512], F32, tag="sumhi")
                    exlo_ps = smp.tile([D, 512], F32, tag="exlops")
                    zps = zpp.tile([2, 512], F32, tag="zps")
                    eps_t = epool.tile([D, CB], BF16, tag="eps",
                                       name=f"eps{b}")

                    nch = (CB + 1023) // 1024
                    if variant == "dmaonly":
                        nch = 1
                    prev = None  # (eps, ylo, nq, first) lagged one chunk
                    for ci in range(nch):
                        t = ci * 1024
                        cw = min(1024, CB - t)
                        nq = cw // 512
                        if variant == "dmaonly":
                            dd = thpool.tile([D, cw], BF16, tag="th")
                            nc.vector.tensor_copy(dd[:], xlo_t[:, t:t + cw])
                            continue
                        first = ci == 0
                        last = ci == nch - 1
                        hp = hpp.tile([D, cw], F32, tag="hp")
                        for q in range(nq):
                            nc.tensor.matmul(
                                hp[:, q * 512:(q + 1) * 512], BDWlo[:],
                                xlo_t[:, t + q * 512:t + (q + 1) * 512],
                                start=True, stop=False)
                            nc.tensor.matmul(
                                hp[:, q * 512:(q + 1) * 512], BDWhi[:],
                                xhi_t[:, t + q * 512:t + (q + 1) * 512],
                                start=False, stop=True)
                        th = thpool.tile([D, cw], BF16, tag="th")
                        nc.scalar.activation(th[:], hp[:], ACTF.Tanh,
                                             bias=biasp[:], scale=1.0)
                        # x-only matmuls cover the tanh latency on PE
                        for q in range(nq if variant != "nopex" else 0):
                            nc.tensor.matmul(
                                sumlo_ps[:], ident[:],
                                xlo_t[:, t + q * 512:t + (q + 1) * 512],
                                start=(first and q == 0),
                                stop=(last and q == nq - 1))
                            nc.tensor.matmul(
                                sumhi_ps[:], ident[:],
                                xhi_t[:, t + q * 512:t + (q + 1) * 512],
                                start=(first and q == 0),
                                stop=(last and q == nq - 1))
                        sps = spp.tile([D, cw], F32, tag="sp")
                        for q in range(nq):
                            nc.tensor.matmul(
                                sps[:, q * 512:(q + 1) * 512], BDctx[:],
                                th[:, q * 512:(q + 1) * 512],
                                start=True, stop=True)
                        # lagged z / exsum-lo matmuls: consume the PREVIOUS
                        # chunk's eps/ylo so PE never waits on this chunk's exp
                        if prev is not None and variant != "nopex":
                            peps, pylo, pm0, pmw, pfirst = prev
                            for q in range(pmw // 512):
                                nc.tensor.matmul(
                                    zps[:], selE[:],
                                    peps[:, pm0 + q * 512:
                                         pm0 + (q + 1) * 512],
                                    start=(pfirst and q == 0), stop=False)
                                nc.tensor.matmul(
                                    exlo_ps[:], ident[:],
                                    pylo[:, q * 512:(q + 1) * 512],
                                    start=(pfirst and q == 0), stop=False)
                            prev = None
                        nc.scalar.activation(eps_t[:, t:t + cw], sps[:],
                                             ACTF.Exp)

                        # DVE ops run on PAIRS of chunks (2048-wide) to halve
                        # DVE op count (per-op DRAIN is comparable to op time)
                        if variant == "nodve":
                            continue
                        if not (last or ci % 2 == 1):
                            continue
                        m0 = (ci // 2) * 2048 if ci % 2 == 1 else t
                        mw = t + cw - m0
                        mi = ci // 2           # merge index
                        pa, pb_ = mi % 2, 1 - mi % 2
                        mfirst = m0 == 0
                        ylo = ypool.tile([D, 2048], BF16, tag="ylo")
                        nc.vector.tensor_tensor(
                            out=ylo[:, 0:mw], in0=xlo_t[:, m0:m0 + mw],
                            in1=eps_t[:, m0:m0 + mw], op=ALU.mult)
                        prev = (eps_t, ylo, m0, mw, mfirst)
                        if mfirst:
                            nc.vector.tensor_copy(
                                maxlo_a[pb_][:, 0:mw], xlo_t[:, m0:m0 + mw])
                            nc.vector.tensor_copy(
                                maxhi_a[pb_][:, 0:mw], xhi_t[:, m0:m0 + mw])
                            nc.vector.tensor_tensor(
                                out=exhi_a[pb_][:, 0:mw],
                                in0=xhi_t[:, m0:m0 + mw],
                                in1=eps_t[:, m0:m0 + mw], op=ALU.mult)
                            if mw < accw:
                                nc.vector.tensor_copy(
                                    maxlo_a[pb_][:, mw:accw],
                                    maxlo_a[pb_][:, 0:accw - mw])
                                nc.vector.tensor_copy(
                                    maxhi_a[pb_][:, mw:accw],
                                    maxhi_a[pb_][:, 0:accw - mw])
                                nc.vector.memset(
                                    exhi_a[pb_][:, mw:accw], 0.0)
                        else:
                            yhi = ypool.tile([D, 2048], BF16, tag="yhi")
                            nc.vector.tensor_tensor(
                                out=yhi[:, 0:mw], in0=xhi_t[:, m0:m0 + mw],
                                in1=eps_t[:, m0:m0 + mw], op=ALU.mult)
                            nc.vector.tensor_tensor(
                                out=maxlo_a[pb_][:, 0:mw],
                                in0=maxlo_a[pa][:, 0:mw],
                                in1=xlo_t[:, m0:m0 + mw], op=ALU.max)
                            nc.vector.tensor_tensor(
                                out=maxhi_a[pb_][:, 0:mw],
                                in0=maxhi_a[pa][:, 0:mw],
                                in1=xhi_t[:, m0:m0 + mw], op=ALU.max)
                            nc.vector.tensor_tensor(
                                out=exhi_a[pb_][:, 0:mw],
                                in0=exhi_a[pa][:, 0:mw], in1=yhi[:, 0:mw],
                                op=ALU.add)
                            if mw < accw:
                                nc.vector.tensor_copy(
                                    maxlo_a[pb_][:, mw:accw],
                                    maxlo_a[pa][:, mw:accw])
                                nc.vector.tensor_copy(
                                    maxhi_a[pb_][:, mw:accw],
                                    maxhi_a[pa][:, mw:accw])
                                nc.vector.tensor_copy(
                                    exhi_a[pb_][:, mw:accw],
                                    exhi_a[pa][:, mw:accw])

                    # flush last merge's lagged z / exsum-lo matmuls
                    if prev is not None and variant not in ("nopex",):
                        peps, pylo, pm0, pmw, pfirst = prev
                        pnq2 = pmw // 512
                        for q in range(pnq2):
                            nc.tensor.matmul(
                                zps[:], selE[:],
                                peps[:, pm0 + q * 512:pm0 + (q + 1) * 512],
                                start=(pfirst and q == 0),
                                stop=(q == pnq2 - 1))
                            nc.tensor.matmul(
                                exlo_ps[:], ident[:],
                                pylo[:, q * 512:(q + 1) * 512],
                                start=(pfirst and q == 0),
                                stop=(q == pnq2 - 1))

                    # ---- block tails ------------------------------------
                    if variant in ("dmaonly", "nodve", "nopex"):
                        if b == 0:
                            for Ct in (sumloC, sumhiC, maxloC, maxhiC,
                                       exloC, exhiC, zC):
                                nc.vector.memset(Ct[:], 1.0)
                        continue
                    nmerge = (nch + 1) // 2
                    fb = 1 - (nmerge - 1) % 2  # written by last merge
                    for (acc, Sdst, op) in (
                        (maxlo_a[fb], maxloS, ALU.max),
                        (maxhi_a[fb], maxhiS, ALU.max),
                        (exhi_a[fb], exhiS, ALU.add),
                    ):
                        w = accw
                        cur = acc
                        while w > 512:
                            half = w // 2
                            dst = fpool.tile([D, half], BF16, tag="fold")
                            nc.vector.tensor_tensor(
                                out=dst[:], in0=cur[:, 0:half],
                                in1=cur[:, half:w], op=op)
                            cur = dst
                            w = half
                        nc.vector.tensor_tensor(
                            out=Sdst[:, b * 256:(b + 1) * 256],
                            in0=cur[:, 0:256], in1=cur[:, 256:512], op=op)

                    # sums: evac psum 512-wide, one fold into the stash
                    for (ps, Sdst) in ((sumlo_ps, sumloS),
                                       (sumhi_ps, sumhiS),
                                       (exlo_ps, exloS)):
                        ev = fpool.tile([D, 512], BF16, tag="sev")
                        nc.scalar.activation(ev[:], ps[:], ACTF.Copy)
                        nc.vector.tensor_tensor(
                            out=Sdst[:, b * 256:(b + 1) * 256],
                            in0=ev[:, 0:256], in1=ev[:, 256:512],
                            op=ALU.add)
                    zev = fpool.tile([2, 512], F32, tag="zev")
                    nc.scalar.activation(zev[:], zps[:], ACTF.Copy)
                    w = 512
                    cur = zev
                    while w > NB:
                        half = w // 2
                        dst = fpool.tile([2, half], F32, tag="zfold")
                        nc.vector.tensor_tensor(
                            out=dst[:], in0=cur[:, 0:half],
                            in1=cur[:, half:w], op=ALU.add)
                        cur = dst
                        w = half
                    nc.vector.tensor_scalar(
                        zC[:, b * NB:(b + 1) * NB], cur[:, 0:NB],
                        1.0, None, ALU.mult)

            # ---- global tail folds: [D, 8 blocks, 256] -> [D, 128] -----
            with tc.tile_pool(name="gfold", bufs=2) as gfp:
                for (S, Cdst, op, fdt) in (
                    (maxloS, maxloC, ALU.max, BF16),
                    (maxhiS, maxhiC, ALU.max, BF16),
                    (exhiS, exhiC, ALU.add, F32),
                    (sumloS, sumloC, ALU.add, F32),
                    (sumhiS, sumhiC, ALU.add, F32),
                    (exloS, exloC, ALU.add, F32),
                ):
                    w = 256
                    cur = S
                    while w > 2 * NB:
                        half = w // 2
                        dst = gfp.tile([D, 8 * half], BF16, tag="gf")
                        cv = cur[:].rearrange("p (s c) -> p s c", s=8)
                        dv = dst[:].rearrange("p (s c) -> p s c", s=8)
                        nc.vector.tensor_tensor(
                            out=dv[:, :, 0:half], in0=cv[:, :, 0:half],
                            in1=cv[:, :, half:w], op=op)
                        cur = dst
                        w = half
                    cv = cur[:].rearrange("p (s c) -> p s c", s=8)
                    Cv = Cdst[:].rearrange("p (s c) -> p s c", s=8)
                    nc.vector.tensor_tensor(
                        out=Cv[:, :, 0:NB], in0=cv[:, :, 0:NB],
                        in1=cv[:, :, NB:2 * NB], op=op)

            # ---- final stage -------------------------------------------
            with (
                tc.tile_pool(name="fin", bufs=1) as fp,
                tc.tile_pool(name="fps", bufs=1, space="PSUM") as fps,
            ):
                # z transpose [2, 128] -> [128, 2]
                zT_ps = fps.tile([128, 2], F32, tag="zT")
                nc.tensor.transpose(zT_ps[:], zC[:], identf[0:2, 0:2])
                zT = fp.tile([128, 2], F32, name="zT")
                nc.scalar.activation(zT[:], zT_ps[:], ACTF.Copy)

                # assemble quadrants via small DMAs: per g-block [128, 128]
                sumT = [fp.tile([D, 128], F32, name=f"sumT{g}")
                        for g in range(2)]
                maxT = [fp.tile([D, 128], BF16, name=f"maxT{g}")
                        for g in range(2)]
                exT = [fp.tile([D, 128], F32, name=f"exT{g}")
                       for g in range(2)]
                for g in range(2):
                    r0 = 64 * g
                    nc.sync.dma_start(sumT[g][0:64, :],
                                      sumloC[r0:r0 + 64, :])
                    nc.sync.dma_start(sumT[g][64:128, :],
                                      sumhiC[r0:r0 + 64, :])
                    nc.sync.dma_start(maxT[g][0:64, :],
                                      maxloC[r0:r0 + 64, :])
                    nc.sync.dma_start(maxT[g][64:128, :],
                                      maxhiC[r0:r0 + 64, :])
                    nc.sync.dma_start(exT[g][0:64, :], exloC[r0:r0 + 64, :])
                    nc.sync.dma_start(exT[g][64:128, :],
                                      exhiC[r0:r0 + 64, :])

                for g in range(2):
                    rc = fp.tile([128, 1], F32, name=f"rc{g}")
                    nc.vector.reciprocal(rc[:], counts_bl[g][:])
                    zt = fp.tile([128, 1], F32, name=f"zt{g}")
                    nc.vector.tensor_sub(zt[:], zT[:, g:g + 1],
                                         zcorr_bl[g][:])
                    rz = fp.tile([128, 1], F32, name=f"rz{g}")
                    nc.vector.reciprocal(rz[:], zt[:])

                    # attn (transposed, unnormalized): outW.T @ exT + outb
                    exbf = fp.tile([128, 128], BF16, name=f"exbf{g}")
                    nc.vector.tensor_copy(exbf[:], exT[g][:])
                    apT_ps = fps.tile([128, 128], F32, tag="apT")
                    nc.tensor.matmul(apT_ps[:], outW_sb[:], exbf[:],
                                     start=True, stop=True)
                    apT = fp.tile([128, 128], F32, name=f"apT{g}")
                    nc.scalar.activation(apT[:], apT_ps[:], ACTF.Identity,
                                         bias=outb_sb[:], scale=1.0)

                    mean_gd = fp.tile([128, 128], F32, name=f"mean{g}")
                    tp = fps.tile([128, 128], F32, tag="tp")
                    nc.tensor.transpose(tp[:], sumT[g][:], identf[:])
                    nc.scalar.activation(mean_gd[:], tp[:], ACTF.Identity,
                                         bias=0.0, scale=rc[:])
                    maxf = fp.tile([128, 128], F32, name=f"maxf{g}")
                    nc.vector.tensor_copy(maxf[:], maxT[g][:])
                    max_gd = fp.tile([128, 128], F32, name=f"maxgd{g}")
                    tp2 = fps.tile([128, 128], F32, tag="tp")
                    nc.tensor.transpose(tp2[:], maxf[:], identf[:])
                    nc.scalar.activation(max_gd[:], tp2[:], ACTF.Identity,
                                         bias=0.0, scale=1.0)
                    attn_gd = fp.tile([128, 128], F32, name=f"attn{g}")
                    tp3 = fps.tile([128, 128], F32, tag="tp")
                    nc.tensor.transpose(tp3[:], apT[:], identf[:])
                    nc.scalar.activation(attn_gd[:], tp3[:], ACTF.Identity,
                                         bias=0.0, scale=rz[:])

                    poolsT_bf = []
                    for nm, gd in (("m", mean_gd), ("x", max_gd),
                                   ("a", attn_gd)):
                        tpp = fps.tile([128, 128], F32, tag="tp")
                        nc.tensor.transpose(tpp[:], gd[:], identf[:])
                        tbf = fp.tile([128, 128], BF16, name=f"p{nm}T{g}")
                        nc.scalar.activation(tbf[:], tpp[:], ACTF.Identity,
                                             bias=0.0, scale=1.0)
                        poolsT_bf.append(tbf)

                    hid_ps = fps.tile([128, 128], F32, tag="hid")
                    for k in range(3):
                        nc.tensor.matmul(
                            hid_ps[:], selW1_sb[:, k * D:(k + 1) * D],
                            poolsT_bf[k][:], start=(k == 0), stop=(k == 2))
                    hid_bf = fp.tile([128, 128], BF16, name=f"hid{g}")
                    nc.scalar.activation(hid_bf[:], hid_ps[:], ACTF.Relu,
                                         bias=selb1_sb[:], scale=1.0)

                    lg_ps = fps.tile([3, 128], F32, tag="lg")
                    nc.tensor.matmul(lg_ps[:], selW2_sb[:], hid_bf[:],
                                     start=True, stop=True)
                    lgT = fp.tile([3, 128], F32, name=f"lgT{g}")
                    nc.scalar.activation(lgT[:], lg_ps[:], ACTF.Identity,
                                         bias=selb2_sb[:], scale=1.0)

                    lg_ps2 = fps.tile([128, 3], F32, tag="lgt")
                    nc.tensor.transpose(lg_ps2[:], lgT[:], identf[0:3, 0:3])
                    lg = fp.tile([128, 3], F32, name=f"lg{g}")
                    nc.scalar.activation(lg[:], lg_ps2[:], ACTF.Identity,
                                         bias=0.0, scale=1.0)

                    m3 = fp.tile([128, 1], F32, name=f"m3{g}")
                    nc.vector.tensor_reduce(m3[:], lg[:],
                                            mybir.AxisListType.X, ALU.max)
                    nm3 = fp.tile([128, 1], F32, name=f"nm3{g}")
                    nc.vector.tensor_scalar(nm3[:], m3[:], -1.0, None,
                                            ALU.mult)
                    ew = fp.tile([128, 3], F32, name=f"ew{g}")
                    den = fp.tile([128, 1], F32, name=f"den{g}")
                    nc.scalar.activation(ew[:], lg[:], ACTF.Exp,
                                         bias=nm3[:], scale=1.0,
                                         accum_out=den[:])
                    rden = fp.tile([128, 1], F32, name=f"rden{g}")
                    nc.vector.reciprocal(rden[:], den[:])
                    w3 = fp.tile([128, 3], F32, name=f"w3{g}")
                    nc.vector.tensor_scalar(w3[:], ew[:], rden[:], None,
                                            ALU.mult)

                    t1 = fp.tile([128, 128], F32, name=f"t1{g}")
                    nc.vector.tensor_scalar(t1[:], mean_gd[:], w3[:, 0:1],
                                            None, ALU.mult)
                    t2 = fp.tile([128, 128], F32, name=f"t2{g}")
                    nc.vector.scalar_tensor_tensor(
                        out=t2[:], in0=max_gd[:], scalar=w3[:, 1:2],
                        in1=t1[:], op0=ALU.mult, op1=ALU.add)
                    out_sb = fp.tile([128, D], F32, name=f"out{g}")
                    nc.vector.scalar_tensor_tensor(
                        out=out_sb[:], in0=attn_gd[:], scalar=w3[:, 2:3],
                        in1=t2[:], op0=ALU.mult, op1=ALU.add)
                    nc.sync.dma_start(outP[g * 128:(g + 1) * 128, :],
                                      out_sb[:])

    nc.compile()
    return nc


# --------------------------------------------------------------------------
# host orchestration
# --------------------------------------------------------------------------

_CACHE = {}
VARIANT = "full"


def _plan(batch, G):
    """Global (core-independent) packing plan from the sorted batch vector."""
    starts = np.searchsorted(batch, np.arange(G + 1))
    counts = np.diff(starts).astype(np.int64)            # [G]
    GPC = G // NCORES
    percore = counts.reshape(NCORES, GPC)                 # [8, 256]
    lmax = percore.max(axis=0)                            # [256] per slot
    lmp = np.maximum(lmax[0::2], lmax[1::2])              # [128] per pair
    order = np.argsort(-lmp, kind="stable")               # pairs sorted desc
    LBs = []
    for b in range(NBLK):
        lb = int(lmp[order[b * NB]])
        lb = max(32, (lb + 31) // 32 * 32)
        LBs.append(lb)
    offs = np.concatenate([[0], np.cumsum([NB * lb for lb in LBs])])
    W = int(offs[-1])
    return starts, counts, lmp, order, tuple(LBs), offs, W


def _col_maps(starts, counts, order, LBs, offs, W, core):
    """Per-core source-node index (into full x) for each column, for A and B
    halves; -1 where padding."""
    GPC = 2 * NPAIR
    g0 = core * GPC
    colA = np.full(W, -1, np.int64)
    colB = np.full(W, -1, np.int64)
    for b in range(NBLK):
        LB = LBs[b]
        o0 = offs[b]
        for j in range(NB):
            p = order[b * NB + j]
            for col, g in ((colA, g0 + 2 * p), (colB, g0 + 2 * p + 1)):
                c = int(counts[g])
                s = int(starts[g])
                dst = o0 + np.arange(c) * NB + j
                col[dst] = np.arange(s, s + c)
    return colA, colB


def _prep_core(x_bf, colA, colB):
    W = colA.shape[0]
    xlo = np.zeros((128, W), dtype=ml_dtypes.bfloat16)
    xhi = np.zeros((128, W), dtype=ml_dtypes.bfloat16)
    mA = colA >= 0
    mB = colB >= 0
    xlo[0:64, mA] = x_bf[colA[mA], 0:64].T
    xlo[64:128, mB] = x_bf[colB[mB], 0:64].T
    xhi[0:64, mA] = x_bf[colA[mA], 64:128].T
    xhi[64:128, mB] = x_bf[colB[mB], 64:128].T
    return xlo, xhi


def _weights_maps(weights):
    (att_W, att_b, att_ctx, out_W, out_b,
     sel_W1, sel_b1, sel_W2, sel_b2) = weights
    BDWlo = np.zeros((128, 128), np.float32)
    BDWlo[0:64, 0:64] = att_W[0:64, :]
    BDWlo[64:128, 64:128] = att_W[0:64, :]
    BDWhi = np.zeros((128, 128), np.float32)
    BDWhi[0:64, 0:64] = att_W[64:128, :]
    BDWhi[64:128, 64:128] = att_W[64:128, :]
    BDctx = np.zeros((128, 128), np.float32)
    BDctx[0:64, 0:64] = att_ctx[:, None]
    BDctx[64:128, 64:128] = att_ctx[:, None]
    selE = np.zeros((128, 2), np.float32)
    selE[0, 0] = 1.0
    selE[64, 1] = 1.0
    bf = ml_dtypes.bfloat16
    return {
        "BDWlo": BDWlo.astype(bf),
        "BDWhi": BDWhi.astype(bf),
        "BDctx": BDctx.astype(bf),
        "biasp": np.concatenate([att_b, att_b]).astype(np.float32)
                  .reshape(128, 1),
        "selE": selE.astype(bf),
        "ident": np.eye(128, dtype=np.float32).astype(bf),
        "identf": np.eye(128, dtype=np.float32),
        "outW": out_W.astype(bf),
        "outb": out_b.astype(np.float32).reshape(128, 1),
        "selW1": sel_W1.astype(bf),
        "selb1": sel_b1.astype(np.float32).reshape(128, 1),
        "selW2": sel_W2.astype(bf),
        "selb2": sel_b2.astype(np.float32).reshape(3, 1),
    }


def prepare(x, batch, att_W, att_b, att_ctx, out_W, out_b,
            sel_W1, sel_b1, sel_W2, sel_b2, num_graphs):
    x = np.asarray(x, dtype=np.float32)
    batch = np.asarray(batch).astype(np.int64)
    G = int(num_graphs)
    assert G == 2048 and x.shape[1] == D

    starts, counts, lmp, order, LBs, offs, W = _plan(batch, G)

    key = ("v2", LBs, W, VARIANT)
    if key not in _CACHE:
        _CACHE[key] = build_nc(LBs, W, VARIANT)
    nc = _CACHE[key]

    weights = tuple(np.asarray(w) for w in
                    (att_W, att_b, att_ctx, out_W, out_b,
                     sel_W1, sel_b1, sel_W2, sel_b2))
    wmap = _weights_maps(weights)

    att_b64 = np.asarray(att_b, np.float64)
    ctx64 = np.asarray(att_ctx, np.float64)
    e_pad = float(np.exp(np.tanh(att_b64) @ ctx64))

    x_bf = x.astype(ml_dtypes.bfloat16)
    in_maps = []
    # device graph order: row i (i<128) = graph 2*order[i] (+ core offset),
    # row 128+i = graph 2*order[i]+1
    gdev = np.empty(2 * NPAIR, np.int64)
    gdev[0:NPAIR] = 2 * order
    gdev[NPAIR:] = 2 * order + 1
    LB_of_pair = np.empty(NPAIR, np.int64)
    for b in range(NBLK):
        LB_of_pair[order[b * NB:(b + 1) * NB]] = LBs[b]
    LB_of_g = np.empty(2 * NPAIR, np.int64)
    LB_of_g[0:NPAIR] = LB_of_pair[order]
    LB_of_g[NPAIR:] = LB_of_pair[order]

    for core in range(NCORES):
        colA, colB = _col_maps(starts, counts, order, LBs, offs, W, core)
        xlo, xhi = _prep_core(x_bf, colA, colB)
        g0 = core * 2 * NPAIR
        cg = np.maximum(counts[g0 + gdev], 1).astype(np.float64)
        # 1e-8 matches the reference's softmax denominator epsilon and keeps
        # rz finite for (astronomically unlikely) empty graphs
        zcorr = (LB_of_g - counts[g0 + gdev]) * e_pad - 1e-8
        im = dict(wmap)
        im["xlo"] = xlo
        im["xhi"] = xhi
        im["counts"] = cg.astype(np.float32).reshape(-1, 1)
        im["zcorr"] = zcorr.astype(np.float32).reshape(-1, 1)
        in_maps.append(im)

    return nc, in_maps, gdev


def _run(x, batch, att_W, att_b, att_ctx, out_W, out_b,
         sel_W1, sel_b1, sel_W2, sel_b2, num_graphs, **spmd_kwargs):
    nc, in_maps, gdev = prepare(
        x, batch, att_W, att_b, att_ctx, out_W, out_b,
        sel_W1, sel_b1, sel_W2, sel_b2, num_graphs)
    res = run_bass_kernel_spmd(nc, in_maps, core_ids=list(range(NCORES)),
                               **spmd_kwargs)
    G = int(num_graphs)
    out = np.empty((G, D), np.float32)
    for core in range(NCORES):
        o = np.asarray(res.results[core]["out"], dtype=np.float32)
        out[core * 2 * NPAIR + gdev] = o
    return out, res


def kernel(**inputs):
    return _run(**inputs)[0]

